# revision 23
# baseline (speedup 1.0000x reference)
"""MeanStdMax pooling kernel for Trainium2 (8 NeuronCores, data-parallel).

Input : hidden_states [16, 13, 512, 768] fp32
Output: [16, 13, 2304] fp32 = concat([sum(seq), std(seq, ddof=1), max(seq)], -1)

Sharding: batch dim 16 -> 2 batches per core (no cross-core communication).

Per-core plan (26 (b,l) pairs, each [512, 768]):
  - DMA each pair as one [128, 4*768] tile; partition p holds seq rows
    4p..4p+3, so every partition is one contiguous 12KB DRAM chunk.  The 16
    DMA engines sustain ~410GB/s aggregate when the sync queue carries ONLY
    input loads (foreign packets in-stream cost ~15% bandwidth), so all
    output DMAs either lag far behind their deps or issue after the loop.
  - sum  : fp32r one-hot-weight matmuls straight off the raw tile; PSUM row
           accumulates pair j's per-hidden sums (partition reduce on the PE).
           PSUM accumulation is split into TWO groups (pairs 0-19 / 20-25)
           so the big stats epilogue runs hidden mid-stream; only a 6-row
           epilogue remains after the final tile.
  - sumsq: ACT Square -> bf16, then bf16 one-hot matmuls into 2nd accumulator.
  - max  : DVE max tree over the 4 seq blocks -> M [128,768], then a
           partition reduce.  gpsimd partition_all_reduce takes ~4us/pair
           mid-stream (SBUF port contention) and runs saturated, so its
           backlog would pace the tail: the LAST pairs (TAILP) instead go
           through PE transposes into a PSUM scratch tile + DVE reduce_max
           over the free axis (both engines have tail slack), a second PE
           transpose fixes the output layout, and one affine DMA emits all
           tail maxes.  The two stats groups SHARE psum banks (the tile
           tracker orders group B's start behind epilogue A's reads) to
           free the banks the transpose scratch needs.
  - last two tiles stream as two half-tiles each so the DVE tree and ACT
    square of the final pair start ~1.5us earlier.
  - epilogue: std = sqrt((sumsq - sum^2/512)/511).
"""

import os
import sys

import numpy as np

for _p in ("/opt/trn_rl_repo", "/root/.axon_site/_ro/trn_rl_repo"):
    if os.path.isdir(_p) and _p not in sys.path:
        sys.path.insert(0, _p)

import concourse.bacc as bacc
import concourse.bass as bass
import concourse.bass_isa as bass_isa
import concourse.masks as masks
import concourse.mybir as mybir
import concourse.tile as tile
from concourse.bass_utils import run_bass_kernel_spmd

N_CORES = 8
B_FULL, L, S, H = 16, 13, 512, 768
B = B_FULL // N_CORES  # 2 batches per core
P = 128
NBLK = S // P  # 4
NPAIR = B * L  # 26
F32 = mybir.dt.float32
F32R = mybir.dt.float32r
BF16 = mybir.dt.bfloat16

GSPLIT = 20              # stats psum groups: pairs [0,20) and [20,26)
SPLIT_TILES = (24, 25)   # stream these pairs as two half-tiles
TAILP = (23, 24, 25)     # partition-max on DVE at the tail (not gpsimd)

_CACHE = {}


def _build():
    if "nc" in _CACHE:
        return _CACHE["nc"]

    nc = bacc.Bacc("TRN2", target_bir_lowering=False, debug=False,
                   num_devices=N_CORES)
    # float32r: same bits as fp32, but satisfies the BIR verifier's
    # "rounded to FP32r" rule so DMA-loaded tiles can feed fp32r matmuls
    # (the fast single-pass fp32 PE mode, ~0.5ns/row vs 1.7 for fp32).
    x = nc.dram_tensor("x", [B, L, S, H], F32R, kind="ExternalInput").ap()
    out = nc.dram_tensor("out", [B, L, 3 * H], F32, kind="ExternalOutput").ap()
    out2 = out.rearrange("b l h -> (b l) h")  # [26, 2304]

    with tile.TileContext(nc) as tc:
        with (
            tc.tile_pool(name="inp", bufs=7) as in_pool,
            tc.tile_pool(name="sq", bufs=4) as sq_pool,
            tc.tile_pool(name="acc", bufs=4) as acc_pool,
            tc.tile_pool(name="mred", bufs=7) as mred_pool,
            tc.tile_pool(name="const", bufs=1) as const_pool,
            tc.tile_pool(name="ep", bufs=1) as ep_pool,
            tc.tile_pool(name="psum", bufs=1, space="PSUM") as psum_pool,
        ):
            # one-hot weight bank: W[:, 26-j : 58-j] is all-ones exactly at
            # local column j.
            W0 = const_pool.tile([P, NPAIR + 32], F32)
            nc.gpsimd.memset(W0[:], 0.0)
            nc.gpsimd.memset(W0[:, NPAIR:NPAIR + 1], 1.0)
            Wr = const_pool.tile([P, NPAIR + 32], F32R)
            nc.vector.tensor_copy(Wr[:], W0[:])
            Wb = const_pool.tile([P, NPAIR + 32], BF16)
            nc.vector.tensor_copy(Wb[:], W0[:])

            Ident = const_pool.tile([P, P], F32)
            masks.make_identity(nc, Ident[:])

            # stats psum tiles; both accumulation groups SHARE them (group B
            # start matmuls are ordered behind epilogue A's psum reads by
            # the tile tracker), freeing psum banks for the tail max path.
            ps0 = {
                "sum_a": psum_pool.tile([32, 512], F32, name="sum_a",
                                        tag="sum_a"),
                "sum_b": psum_pool.tile([32, 256], F32, name="sum_b",
                                        tag="sum_b"),
                "sq_a": psum_pool.tile([32, 512], F32, name="sq_a",
                                       tag="sq_a"),
                "sq_b": psum_pool.tile([32, 256], F32, name="sq_b",
                                       tag="sq_b"),
            }
            ps = [ps0, ps0]
            # tail max scratch: TP holds a pair's M transposed (hidden on
            # partitions), TP2 the layout-fixing transpose of the result.
            TP = psum_pool.tile([P, H], F32, name="TP", tag="TP")
            TP2 = psum_pool.tile([8, P], F32, name="TP2", tag="TP2")
            # row c of a pair's slice = maxes of hidden 128c..128c+127
            stageT = ep_pool.tile([8, P * len(TAILP)], F32)

            def grp(j):
                return 0 if j < GSPLIT else 1

            def is_start(j):
                return j in (0, GSPLIT)

            def is_stop(j):
                return j in (GSPLIT - 1, NPAIR - 1)

            def local(j):
                return j - (0 if j < GSPLIT else GSPLIT)

            def emit_epilogue(g, lo, hi):
                # std = sqrt((sumsq - sum^2/n)/(n-1)); also stages sums.
                # psum rows are group-local; the DMA is issued separately
                # (after the loop; see module docstring).
                n = hi - lo
                stats = ep_pool.tile([n, 2 * H], F32, tag=f"stats{g}")
                nc.scalar.copy(stats[:, 0:512], ps[g]["sum_a"][0:n])
                nc.scalar.copy(stats[:, 512:768], ps[g]["sum_b"][0:n])
                # sum^2/n on ACT: Square(x/sqrt(n))
                sum2 = ep_pool.tile([n, H], F32, tag=f"sum2{g}")
                nc.scalar.activation(sum2[:], stats[:, 0:H],
                                     mybir.ActivationFunctionType.Square,
                                     scale=1.0 / float(np.sqrt(S)))
                var = ep_pool.tile([n, H], F32, tag=f"var{g}")
                nc.vector.tensor_tensor(var[:, 0:512], ps[g]["sq_a"][0:n],
                                        sum2[:, 0:512],
                                        op=mybir.AluOpType.subtract)
                nc.vector.tensor_tensor(var[:, 512:768], ps[g]["sq_b"][0:n],
                                        sum2[:, 512:768],
                                        op=mybir.AluOpType.subtract)
                nc.scalar.activation(stats[:, H:2 * H], var[:],
                                     mybir.ActivationFunctionType.Sqrt,
                                     scale=1.0 / (S - 1))
                return stats

            # PE runs one pair behind for sq matmuls so its per-iteration
            # work only depends on data from iteration j-1.
            pending = None  # (j, Q_tile)
            # max-out DMAs lag 5 pairs: their gpsimd Mred dep resolves ~12us
            # before they issue, so they never stall the sync queue in front
            # of input prefetch DMAs.
            max_outs = []

            def flush_max_outs(keep):
                while len(max_outs) > keep:
                    jj, mred = max_outs.pop(0)
                    nc.sync.dma_start(out2[jj:jj + 1, 2 * H:3 * H],
                                      mred[0:1, :])

            def emit_tail(j, Q):
                g = grp(j)
                first, last = is_start(j), is_stop(j)
                lj = local(j)
                wjb = Wb[:, NPAIR - lj:NPAIR - lj + 32]
                Qv = Q[:].rearrange("p (n h) -> p n h", h=H)
                for blk in range(NBLK):
                    nc.tensor.matmul(
                        ps[g]["sq_a"][:], wjb, Qv[:, blk, 0:512],
                        start=first and blk == 0, stop=last and blk == NBLK - 1)
                    nc.tensor.matmul(
                        ps[g]["sq_b"][:], wjb, Qv[:, blk, 512:768],
                        start=first and blk == 0, stop=last and blk == NBLK - 1)

            stats_a = None
            tail_r6 = []

            for j in range(NPAIR):
                b, l = divmod(j, L)
                g = grp(j)
                first, last = is_start(j), is_stop(j)

                if j == GSPLIT:
                    # group A psum must fully retire (last sq matmuls + the
                    # epilogue's psum reads) BEFORE group B's start matmuls
                    # reuse the shared banks.
                    if pending is not None:
                        emit_tail(*pending)
                        pending = None
                    stats_a = emit_epilogue(0, 0, GSPLIT)

                T = in_pool.tile([P, NBLK * H], F32R)
                Tr = T[:].rearrange("p (n h) -> p n h", h=H)
                # partition p <- seq rows 4p..4p+3: contiguous 12KB chunks;
                # the seq->(p,i) mapping is irrelevant to sum/max/sumsq.
                src = x[b, l].rearrange("(p n) h -> p n h", n=NBLK)
                if j in SPLIT_TILES:
                    nc.sync.dma_start(Tr[:, 0:2, :], src[:, 0:2, :])
                    nc.sync.dma_start(Tr[:, 2:4, :], src[:, 2:4, :])
                else:
                    nc.sync.dma_start(T[:], src)
                Tv = T[:].bitcast(F32).rearrange("p (n h) -> p n h", h=H)

                # ---- sums: fp32r one-hot matmuls straight off the raw tile ----
                lj = local(j)
                wjr = Wr[:, NPAIR - lj:NPAIR - lj + 32]
                for blk in range(NBLK):
                    nc.tensor.matmul(
                        ps[g]["sum_a"][:], wjr, Tr[:, blk, 0:512],
                        start=first and blk == 0, stop=last and blk == NBLK - 1)
                    nc.tensor.matmul(
                        ps[g]["sum_b"][:], wjr, Tr[:, blk, 512:768],
                        start=first and blk == 0, stop=last and blk == NBLK - 1)

                # ---- max tree on DVE (final level bf16) ----
                m2 = acc_pool.tile([P, 2 * H], F32, tag="m2")
                m2v = m2[:].rearrange("p (n h) -> p n h", h=H)
                if j in SPLIT_TILES:
                    nc.vector.tensor_tensor(
                        m2v[:, 0, :], Tv[:, 0, :], Tv[:, 1, :],
                        op=mybir.AluOpType.max)
                    nc.vector.tensor_tensor(
                        m2v[:, 1, :], Tv[:, 2, :], Tv[:, 3, :],
                        op=mybir.AluOpType.max)
                else:
                    nc.vector.tensor_tensor(
                        m2v, Tv[:, 0:2, :], Tv[:, 2:4, :],
                        op=mybir.AluOpType.max)
                if j not in TAILP:
                    M = acc_pool.tile([P, H], BF16, tag="M")
                    nc.vector.tensor_tensor(M[:], m2v[:, 0, :], m2v[:, 1, :],
                                            op=mybir.AluOpType.max)
                    # partition all-reduce for max on the otherwise idle
                    # gpsimd; its serial throughput is the only reason the
                    # tail pairs take the DVE path instead.
                    Mred = mred_pool.tile([P, H], F32, tag="Mred")
                    nc.gpsimd.partition_all_reduce(
                        Mred[:], M[:], channels=P,
                        reduce_op=bass_isa.ReduceOp.max)
                    max_outs.append((j, Mred))
                else:
                    Mf = acc_pool.tile([P, H], F32, tag="Mf")
                    nc.vector.tensor_tensor(Mf[:], m2v[:, 0, :], m2v[:, 1, :],
                                            op=mybir.AluOpType.max)
                    t = j - TAILP[0]
                    for c in range(6):
                        nc.tensor.transpose(TP[:, P * c:P * (c + 1)],
                                            Mf[:, P * c:P * (c + 1)],
                                            Ident[:])
                    R6 = acc_pool.tile([P, 6], F32, tag="R6")
                    nc.vector.reduce_max(
                        R6[:], TP[:].rearrange("p (c e) -> p c e", e=P),
                        axis=mybir.AxisListType.X)
                    tail_r6.append((t, R6))

                # ---- squares in bf16 on ACT ----
                Q = sq_pool.tile([P, NBLK * H], BF16)
                if j in SPLIT_TILES:
                    nc.scalar.activation(Q[:, 0:2 * H],
                                         T[:, 0:2 * H].bitcast(F32),
                                         mybir.ActivationFunctionType.Square)
                    nc.scalar.activation(Q[:, 2 * H:4 * H],
                                         T[:, 2 * H:4 * H].bitcast(F32),
                                         mybir.ActivationFunctionType.Square)
                else:
                    nc.scalar.activation(Q[:], T[:].bitcast(F32),
                                         mybir.ActivationFunctionType.Square)

                if pending is not None:
                    emit_tail(*pending)
                pending = (j, Q)
                flush_max_outs(keep=5)

            emit_tail(*pending)
            # ---- tail: everything below is after all input DMAs ----
            nc.sync.dma_start(out2[0:GSPLIT, 0:2 * H], stats_a[:])
            flush_max_outs(keep=0)
            stats_b = emit_epilogue(1, GSPLIT, NPAIR)
            nc.sync.dma_start(out2[GSPLIT:NPAIR, 0:2 * H], stats_b[:])
            # tail max emit: layout-fix transpose per pair ([128,6]->[6,128],
            # row c = hidden 128c..), stage rows, one affine DMA.
            for t, R6 in tail_r6:
                nc.tensor.transpose(TP2[0:6, :], R6[:], Ident[:])
                nc.scalar.copy(stageT[0:6, P * t:P * (t + 1)], TP2[0:6, :])
            nc.sync.dma_start(
                out2[TAILP[0]:NPAIR, 2 * H:3 * H].rearrange(
                    "t (c e) -> c t e", e=P),
                stageT[0:6, :].rearrange("c (t e) -> c t e", e=P))

    nc.compile()
    _CACHE["nc"] = nc
    return nc


def _run(hidden_states: np.ndarray, trace: bool = False):
    nc = _build()
    x = np.ascontiguousarray(np.asarray(hidden_states, dtype=np.float32))
    assert x.shape == (B_FULL, L, S, H), x.shape
    in_maps = [{"x": x[c * B:(c + 1) * B]} for c in range(N_CORES)]
    res = run_bass_kernel_spmd(nc, in_maps, core_ids=list(range(N_CORES)),
                               trace=trace)
    out = np.empty((B_FULL, L, 3 * H), dtype=np.float32)
    for c in range(N_CORES):
        out[c * B:(c + 1) * B] = res.results[c]["out"]
    return out, res


def kernel(hidden_states: np.ndarray) -> np.ndarray:
    out, _ = _run(hidden_states)
    return out
